# revision 48
# baseline (speedup 1.0000x reference)
"""NonLocalBlock (nn_NonLocalBlock_80221399155245) — Trainium2 Bass kernel.

Sharding: data-parallel over batch B=8, one batch item per NeuronCore.
Per-core pipeline (xf = x[b] as [C=256, N=4096]):
  theta = Wq @ xf, phi = Wk @ xf        [I=128, N]  (bf16, I-major)
  gT    = (Wg @ xf).T                   (N-major 128-chunks, fp8e4)
  per 256-column n-block:
    logits^T[m, n] = phi_m.T @ theta_n  (PE bf16, 32 m-chunks -> PSUM)
    P^T = exp(logits^T / sqrt(I))       (ACT, PSUM -> SBUF fp8e4)
    denom = colsum(P^T)                 (PE fp8 DoubleRow ones-matmul,
                                         chunk-pairs, PSUM-accumulated)
    outT  = sum_m gT_m.T @ P^T_m        (PE fp8 DoubleRow chunk-pairs: K=256
                                         per instruction, 2x bf16 rate)
    on    = outT * (1/denom)            (gpsimd partition_broadcast + DVE)
    y     = Wo @ on                     (PE bf16) -> drained with fused
                                         per-channel bn_stats (DVE)
  Sync-BN stats cross 8 cores WITHOUT the ~90us ncfw AllReduce data phase:
  a rendezvous-only AllReduce fires at kernel start (hidden under the loop,
  proves every peer passed its NRT sema_reset), then the [128,4] stats go
  out as ONE remote-DMA broadcast per core (slot <core id> of every peer's
  SBUF mailbox, 8-way jump table) and each core reduces its mailbox.
  Everything after that wait lives outside the TileContext (the tile
  scheduler cannot model remotely-incremented semaphores) with hand-rolled
  semaphore gates -- including write-ack gates between same-engine
  dependent tiny ops, which the engines do NOT interlock.
  Epilogue: BN affine folded with the SE sigmoid gate into per-channel
  A,B; y = A*ysb + B + x via fused affine_then_add; outputs DMA per slice.
"""

import numpy as np
import ml_dtypes
import concourse.bass as bass
import concourse.tile as tile
from concourse import bacc, mybir
from concourse.bass_utils import run_bass_kernel_spmd

F32 = mybir.dt.float32
BF16 = mybir.dt.bfloat16
F32R = mybir.dt.float32r
FP8 = mybir.dt.float8e4
DR = mybir.MatmulPerfMode.DoubleRow
AF = mybir.ActivationFunctionType
ALU = mybir.AluOpType

C = 256     # channels
I = 128     # inter channels
R = 64      # SE reduction
P = 128     # SBUF partitions
B = 8       # batch == cores
H = W = 64
N = H * W   # 4096 pixels
NB = 256    # n-block columns
CHUNK_GROUP = 4   # logits chunks per exp-activation group


def _build(n_cores=B, nn=N, nb=NB, chunk_group=CHUNK_GROUP, total_pixels=None):
    M = nn // P
    NBLK = nn // nb
    GRP = M // chunk_group
    assert M % chunk_group == 0
    if total_pixels is None:
        total_pixels = n_cores * nn
    sm_scale = float(1.0 / np.sqrt(np.float32(I)))

    nc = bacc.Bacc("TRN2", target_bir_lowering=False, debug=False,
                   num_devices=n_cores)

    x_d = nc.declare_dram_parameter("x", [C, nn], F32, isOutput=False)
    wq_d = nc.declare_dram_parameter("wq_t", [C, I], BF16, isOutput=False)
    wk_d = nc.declare_dram_parameter("wk_t", [C, I], BF16, isOutput=False)
    wg_d = nc.declare_dram_parameter("wg_t", [C, I], BF16, isOutput=False)
    wo_d = nc.declare_dram_parameter("wo_t", [I, C], BF16, isOutput=False)
    fc1w_d = nc.declare_dram_parameter("fc1_wt", [C, R], F32, isOutput=False)
    fc1b_d = nc.declare_dram_parameter("fc1_b", [R], F32, isOutput=False)
    fc2w_d = nc.declare_dram_parameter("fc2_wt", [R, C], F32, isOutput=False)
    fc2bn_d = nc.declare_dram_parameter("fc2_bn", [C], F32, isOutput=False)
    gam_d = nc.declare_dram_parameter("bn_gamma", [C], F32, isOutput=False)
    bet_d = nc.declare_dram_parameter("bn_beta", [C], F32, isOutput=False)
    out_d = nc.declare_dram_parameter("out", [C, nn], F32, isOutput=True)

    # rendezvous-only collective buffers (values unused); the AllReduce is
    # kicked at kernel start so its ~90us ncfw latency hides under the main
    # loop, and its completion proves every core passed the NRT preamble
    # (sema_reset) — making raw cross-core remote-sem bumps safe afterwards.
    rdv_in = nc.dram_tensor("rdv_in", [P, 1], F32)
    rdv_out = nc.dram_tensor("rdv_out", [P, 1], F32,
                             addr_space="Shared" if n_cores > 4 else "Local")
    rsems = [nc.alloc_semaphore("bn_stats_rsem0")]
    lsem = nc.alloc_semaphore("bn_stats_lsem")
    sem_ps = nc.alloc_semaphore("epi_prep")
    sem_vs = nc.alloc_semaphore("epi_v")
    sem_ss = nc.alloc_semaphore("epi_s")
    sem_vd = nc.alloc_semaphore("epi_vd")
    sem_ds = nc.alloc_semaphore("epi_dma")
    sem_bn = nc.alloc_semaphore("epi_bn")
    sem_st = nc.alloc_semaphore("epi_stats")
    sem_rd = nc.alloc_semaphore("epi_reduce")

    # raw (non-tile) SBUF for everything the post-TileContext epilogue
    # touches — tile-pool APs cannot be referenced outside the TileContext
    # (they stay symbolic). In-tc ops may write these; cross-engine ordering
    # against the raw section is provided by the tc-exit barrier (or by the
    # explicit sem_xl waits for the input loads, which in-tc readers need).
    mb_raw = nc.alloc_sbuf_tensor("mb_raw", [P, 8, 16], F32)
    _pad = nc.alloc_sbuf_tensor("mb_pad", [P, 16], F32)
    acc_raw = [nc.alloc_sbuf_tensor(f"acc_raw{r}", [P, 4], F32) for r in range(4)]
    bn_sc = nc.alloc_sbuf_tensor("bn_sc", [P, 14], F32)
    stats_raw = nc.alloc_sbuf_tensor("stats_raw", [P, 4], F32)
    xres = nc.alloc_sbuf_tensor("xres_raw", [P, 2, nn], F32)
    ysb_all = nc.alloc_sbuf_tensor("ysb_raw", [P, 2, nn], F32)
    gam = nc.alloc_sbuf_tensor("gam_raw", [P, 2], F32)
    bet = nc.alloc_sbuf_tensor("bet_raw", [P, 2], F32)
    chw = nc.alloc_sbuf_tensor("chw_raw", [P, 2], F32)

    with tile.TileContext(nc) as tc:
        import contextlib
        with contextlib.ExitStack() as stack:
            sing = stack.enter_context(tc.tile_pool(name="sing", bufs=1))

            xf32 = [sing.tile([P, nn], F32, tag=f"xf32_{cc}", name=f"xf32_{cc}")
                    for cc in range(2)]
            xbf = [sing.tile([P, nn], BF16, tag=f"xbf_{cc}", name=f"xbf_{cc}")
                   for cc in range(2)]
            theta = sing.tile([P, nn], BF16, tag="theta", name="theta")
            phi = sing.tile([P, nn], BF16, tag="phi", name="phi")
            gT = sing.tile([P, M, I], FP8, tag="gT", name="gT")
            bnst = [sing.tile([P, NBLK, 6], F32, tag=f"bnst_{cc}", name=f"bnst_{cc}")
                    for cc in range(2)]

            wq = sing.tile([P, 2, I], BF16, tag="wq", name="wq")
            wk = sing.tile([P, 2, I], BF16, tag="wk", name="wk")
            wg = sing.tile([P, 2, I], BF16, tag="wg", name="wg")
            wo = sing.tile([P, 2, P], BF16, tag="wo", name="wo")
            fc1w = sing.tile([P, 2, R], F32, tag="fc1w", name="fc1w")
            fc1b = sing.tile([R, 1], F32, tag="fc1b", name="fc1b")
            fc2w = sing.tile([R, 2, P], F32, tag="fc2w", name="fc2w")
            fc2bn = sing.tile([P, 2], F32, tag="fc2bn", name="fc2bn")
            ones_col = sing.tile([P, 1], F32, tag="ones_col", name="ones_col")
            ones_row = sing.tile([1, P], F32, tag="ones_row", name="ones_row")

            ones2 = sing.tile([P, 2, 32], FP8, tag="ones2", name="ones2")
            nc.vector.memset(ones_col, 1.0)
            nc.vector.memset(ones_row, 1.0)
            nc.vector.memset(ones2, 1.0)

            # fire the rendezvous collective first thing on gpsimd
            nc.gpsimd.collective_compute(
                "AllReduce", ALU.add,
                replica_groups=[list(range(n_cores))],
                ins=[rdv_in[:]], outs=[rdv_out[:]])

            for cc in range(2):
                nc.sync.dma_start(out=xf32[cc], in_=x_d[cc * P:(cc + 1) * P, :])
                # second, untracked copy of x for the raw epilogue's residual
                # read (raw-dst DMA; tc-exit drain+barrier orders it)
                nc.sync.dma_start(out=xres[:, cc, :],
                                  in_=x_d[cc * P:(cc + 1) * P, :])
            nc.sync.dma_start(out=wq, in_=wq_d.rearrange("(a p) i -> p a i", p=P))
            nc.sync.dma_start(out=wk, in_=wk_d.rearrange("(a p) i -> p a i", p=P))
            nc.sync.dma_start(out=wg, in_=wg_d.rearrange("(a p) i -> p a i", p=P))
            nc.sync.dma_start(out=wo, in_=wo_d.rearrange("i (a c) -> i a c", a=2))
            nc.sync.dma_start(out=fc1w, in_=fc1w_d.rearrange("(a p) r -> p a r", p=P))
            nc.sync.dma_start(out=fc1b, in_=fc1b_d[:, None])
            nc.sync.dma_start(out=fc2w, in_=fc2w_d.rearrange("r (a c) -> r a c", a=2))
            nc.sync.dma_start(out=fc2bn, in_=fc2bn_d.rearrange("(a p) -> p a", p=P))
            with nc.allow_non_contiguous_dma(reason="tiny [128,2] param loads"):
                nc.sync.dma_start(out=gam[:, :],
                                  in_=gam_d.rearrange("(a p) -> p a", p=P))
                nc.sync.dma_start(out=bet[:, :],
                                  in_=bet_d.rearrange("(a p) -> p a", p=P))

            nc.vector.tensor_copy(xbf[0][:], xf32[0][:])
            nc.scalar.copy(xbf[1][:], xf32[1][:])

            # ---- prologue: QKV projections + SE ----
            with tc.tile_pool(name="proj_ps", bufs=4, space="PSUM") as pps, \
                 tc.tile_pool(name="se_ps", bufs=2, space="PSUM") as seps:
                NCH = nn // 512
                for (wt, dst) in ((wq, theta), (wk, phi)):
                    for t in range(NCH):
                        ps = pps.tile([P, 512], F32, tag="proj", name="proj")
                        for cc in range(2):
                            nc.tensor.matmul(
                                ps[:], wt[:, cc, :],
                                xbf[cc][:, t * 512:(t + 1) * 512],
                                start=(cc == 0), stop=(cc == 1))
                        if t % 2 == 0:
                            nc.scalar.copy(dst[:, t * 512:(t + 1) * 512], ps[:])
                        else:
                            nc.vector.tensor_copy(dst[:, t * 512:(t + 1) * 512], ps[:])

                for q in range(M // 4):
                    ps = pps.tile([P, 4, I], F32, tag="proj", name="proj")
                    for j in range(4):
                        mj = q * 4 + j
                        for cc in range(2):
                            nc.tensor.matmul(
                                ps[:, j, :], xbf[cc][:, mj * P:(mj + 1) * P],
                                wg[:, cc, :], start=(cc == 0), stop=(cc == 1))
                    dst = gT[:, q * 4:(q + 1) * 4, :]
                    src = ps[:, :, :]
                    if q % 2 == 0:
                        nc.scalar.copy(dst, src)
                    else:
                        nc.vector.tensor_copy(dst, src)

                pooled = sing.tile([P, 2], F32, tag="pooled", name="pooled")
                for cc in range(2):
                    nc.vector.reduce_sum(pooled[:, cc:cc + 1], xf32[cc][:],
                                         axis=mybir.AxisListType.X)
                hps = seps.tile([R, 1], F32, tag="se", name="se_h")
                for cc in range(2):
                    nc.tensor.matmul(hps[:], fc1w[:, cc, :], pooled[:, cc:cc + 1],
                                     start=(cc == 0), stop=(cc == 1))
                hsb = sing.tile([R, 1], F32, tag="hsb", name="hsb")
                nc.scalar.activation(hsb[:], hps[:], AF.Relu, bias=fc1b[:])
                for cc in range(2):
                    zps = seps.tile([P, 1], F32, tag="se2", name="se_z")
                    nc.tensor.matmul(zps[:], fc2w[:, cc, :], hsb[:],
                                     start=True, stop=True)
                    esb = sing.tile([P, 1], F32, tag=f"esb_{cc}", name=f"esb_{cc}")
                    nc.scalar.activation(esb[:], zps[:], AF.Exp,
                                         bias=fc2bn[:, cc:cc + 1], scale=-1.0)
                    nc.vector.tensor_scalar_add(esb[:], esb[:], 1.0)
                    nc.vector.reciprocal(chw[:, cc:cc + 1], esb[:])

            # ---- main attention loop ----
            with tc.tile_pool(name="lg", bufs=2, space="PSUM") as lg, \
                 tc.tile_pool(name="outTp", bufs=2, space="PSUM") as outTp, \
                 tc.tile_pool(name="aux_ps", bufs=1, space="PSUM") as aux, \
                 tc.tile_pool(name="pTp", bufs=3) as pTp, \
                 tc.tile_pool(name="smalls", bufs=3) as smalls:

                cs_ps = aux.tile([32, nb], F32, tag="cs", name="cs")
                ypj_ps = aux.tile([P, 2, nb], F32, tag="ypj", name="ypj")

                # wo + drain of block kb are emitted inside block kb+1's
                # group loop: the wo matmul waits ~3us on the DVE/gpsimd
                # normalization chain, and emitting it at the block boundary
                # head-of-line-blocks the next block's logits in the in-order
                # PE queue, starving the exp pipeline.
                pending = []

                def _finish():
                    pkb, pon = pending.pop()
                    for cc in range(2):
                        nc.tensor.matmul(ypj_ps[:, cc, :], wo[:, cc, :],
                                         pon[:], start=True, stop=True)
                    # single drain reads the WHOLE ypj PSUM bank so no DVE
                    # read overlaps the other half's PE write in the same
                    # bank (fatal hazard); per-channel stats via bn_stats
                    nc.vector.tensor_scalar(
                        ysb_all[:, :, pkb * nb:(pkb + 1) * nb], ypj_ps[:, :, :],
                        1.0, None, ALU.mult)
                    for cc in range(2):
                        nc.vector.bn_stats(
                            out=bnst[cc][:, pkb, :],
                            in_=ysb_all[:, cc, pkb * nb:(pkb + 1) * nb])

                for kb in range(NBLK):
                    th_sl = theta[:, kb * nb:(kb + 1) * nb]
                    pT = pTp.tile([P, M, nb], FP8, tag="pT", name="pT")
                    outT_ps = outTp.tile([P, nb], F32, tag="outT", name="outT")

                    # den (fp8 DoubleRow ones-matmul colsums, PSUM row) and
                    # outT accumulate pair-wise, interleaved one group behind
                    # the exp that produces their P^T chunks: the PE then
                    # never dumps a block-boundary matmul lump that starves
                    # the ACT exp pipeline.
                    def _pair(c2):
                        nc.tensor.matmul(
                            cs_ps[:], ones2[:, :, :],
                            pT[:, 2 * c2:2 * c2 + 2, :],
                            start=(c2 == 0), stop=(c2 == M // 2 - 1),
                            perf_mode=DR)
                        nc.tensor.matmul(
                            outT_ps[:], gT[:, 2 * c2:2 * c2 + 2, :],
                            pT[:, 2 * c2:2 * c2 + 2, :],
                            start=(c2 == 0), stop=(c2 == M // 2 - 1),
                            perf_mode=DR)

                    for g in range(GRP):
                        lgt = lg.tile([P, chunk_group, nb], F32, tag="lg", name="lg")
                        for j in range(chunk_group):
                            mj = g * chunk_group + j
                            nc.tensor.matmul(
                                lgt[:, j, :], phi[:, mj * P:(mj + 1) * P], th_sl,
                                start=True, stop=True)
                        nc.scalar.activation(
                            pT[:, g * chunk_group:(g + 1) * chunk_group, :],
                            lgt[:, :, :],
                            AF.Exp, scale=sm_scale)
                        if g == 2 and pending:
                            _finish()
                        if g >= 1:
                            _pair(2 * (g - 1))
                            _pair(2 * (g - 1) + 1)
                    _pair(2 * (GRP - 1))
                    _pair(2 * (GRP - 1) + 1)

                    cs_sb = smalls.tile([1, nb], F32, tag="cs_sb", name="cs_sb")
                    nc.vector.tensor_copy(cs_sb[:], cs_ps[0:1, :])
                    inv = smalls.tile([1, nb], F32, tag="inv", name="inv")
                    nc.vector.reciprocal(inv[:], cs_sb[:])
                    bc_sb = smalls.tile([P, nb], F32, tag="bc_sb", name="bc_sb")
                    nc.gpsimd.partition_broadcast(bc_sb[:], inv[:])

                    on_sb = smalls.tile([P, nb], BF16, tag="on_sb", name="on_sb")
                    nc.vector.tensor_tensor(on_sb[:], outT_ps[:], bc_sb[:], ALU.mult)
                    pending.append((kb, on_sb))
                _finish()

            # ---- epilogue: sync-BN via remote-DMA stats exchange ----
            # One-shot XOR all-to-all: 7 broadcasts (k = 1..7), each sending
            # my stats to peer me^k's mailbox slot k (XOR-relative rdests need
            # no core id; receiver slot k holds sender r^k). The descriptor
            # preps are generated mid-loop on the idle gpsimd and the trigger
            # fires as soon as the stats are committed (sem_st), so the data
            # flight overlaps the TileContext-exit drain/barrier. Only safe
            # after the rendezvous collective completed (gpsimd reads rdv_out
            # first), which proves all peers passed their NRT sema_reset.
            with tc.tile_pool(name="epi", bufs=2) as epi:
                rdv_chk = sing.tile([P, 1], F32, tag="rdv_chk", name="rdv_chk")
                nc.gpsimd.dma_start(out=rdv_chk[:], in_=rdv_out[:])
                stats_t = sing.tile([P, 4], F32, tag="stats_t", name="stats_t")

                for cc in range(2):
                    mv = epi.tile([P, 2], F32, tag="mv", name="mv")
                    nc.vector.bn_aggr(out=mv[:], in_=bnst[cc][:, :, :])
                    # sum = mean*nn ; sumsq = (var + mean^2)*nn
                    nc.vector.tensor_scalar_mul(stats_t[:, cc:cc + 1],
                                                mv[:, 0:1], float(nn))
                    m2 = epi.tile([P, 1], F32, tag="m2", name="m2")
                    nc.vector.tensor_tensor(m2[:], mv[:, 0:1], mv[:, 0:1],
                                            ALU.mult)
                    nc.vector.tensor_tensor(m2[:], mv[:, 1:2], m2[:], ALU.add)
                    nc.vector.tensor_scalar_mul(stats_t[:, 2 + cc:3 + cc],
                                                m2[:], float(nn))
                # raw copy for the raw section; the tc-exit barrier orders
                # it before the raw gpsimd trigger reads it.
                nc.vector.tensor_copy(stats_raw[:, :], stats_t[:, :])

    # ---- raw (non-tile) epilogue -------------------------------------
    # The tile scheduler cannot model waits on remotely-incremented
    # semaphores (single-core sim -> deadlock), hence everything past the
    # mailbox wait lives outside the TileContext with hand-rolled syncs.
    # Every same-engine consumer of a just-written tiny tile waits for the
    # producer's write-ack (then_inc fires post-commit) — engines do NOT
    # interlock posted writes against their own subsequent reads.
    V, S, G, Y = nc.vector, nc.scalar, nc.gpsimd, nc.sync
    # one 8-destination broadcast (single SWDGE group, all 16 lanes) into
    # slot <my id> of every core's mailbox; the 8-way jump table supplies
    # the compile-time slot address per core.
    pid = G.partition_id()
    for k in G.Switch(pid, n_cores):
        G.remote_dma_broadcast(
            out_ap=mb_raw[:, k, 0:4], in_ap=stats_raw[:, :],
            remote_sem=rsems[0], local_sem=lsem,
            rdests=[(0, j) for j in range(n_cores)]).then_inc(sem_ps, 1)
    G.wait_ge(sem_ps, 1)
    G.trigger_dma(count=1)
    V.wait_ge(rsems[0], 16)            # 8 senders x 2 lanes landed
    mb = lambda k: mb_raw[:, k, 0:4]
    V.tensor_tensor(acc_raw[0][:, :], mb(0), mb(1), ALU.add).then_inc(sem_rd, 1)
    V.tensor_tensor(acc_raw[1][:, :], mb(2), mb(3), ALU.add).then_inc(sem_rd, 1)
    V.tensor_tensor(acc_raw[2][:, :], mb(4), mb(5), ALU.add).then_inc(sem_rd, 1)
    V.tensor_tensor(acc_raw[3][:, :], mb(6), mb(7), ALU.add).then_inc(sem_rd, 1)
    V.wait_ge(sem_rd, 2)
    V.tensor_tensor(acc_raw[0][:, :], acc_raw[0][:, :], acc_raw[1][:, :],
                    ALU.add).then_inc(sem_rd, 1)
    V.wait_ge(sem_rd, 4)
    V.tensor_tensor(acc_raw[2][:, :], acc_raw[2][:, :], acc_raw[3][:, :],
                    ALU.add).then_inc(sem_rd, 1)
    V.wait_ge(sem_rd, 6)
    V.tensor_tensor(acc_raw[1][:, :], acc_raw[0][:, :], acc_raw[2][:, :],
                    ALU.add).then_inc(sem_vs, 3)
    stats_g = acc_raw[1]

    # BN affine coefficients, both channel-halves vectorized as [P, 2].
    # bn_sc cols: 0:2 mean | 2:4 ex2/Bt | 4:6 var | 6:8 ln/g1 | 8:10 istd
    #             10:12 A | 12:14 B
    # Every same-engine consumer of a just-written tiny tile must wait for
    # the producer's write-ack (then_inc fires post-commit) — engines do NOT
    # interlock posted writes against their own subsequent reads, and these
    # [P,2] ops are shorter than the SBUF write-ack latency.
    inv_np = 1.0 / float(total_pixels)
    V.wait_ge(sem_vs, 3)
    V.tensor_scalar_mul(bn_sc[:, 0:2], stats_g[:, 0:2],
                        inv_np).then_inc(sem_bn, 1)
    V.tensor_scalar_mul(bn_sc[:, 2:4], stats_g[:, 2:4],
                        inv_np).then_inc(sem_bn, 1)
    V.wait_ge(sem_bn, 1)
    V.tensor_tensor(bn_sc[:, 4:6], bn_sc[:, 0:2], bn_sc[:, 0:2],
                    ALU.mult).then_inc(sem_bn, 1)
    V.wait_ge(sem_bn, 3)
    V.tensor_tensor(bn_sc[:, 4:6], bn_sc[:, 2:4], bn_sc[:, 4:6],
                    ALU.subtract).then_inc(sem_bn, 1)
    V.wait_ge(sem_bn, 4)
    V.tensor_scalar_add(bn_sc[:, 4:6], bn_sc[:, 4:6], 1e-5).then_inc(sem_vs, 1)
    V.wait_ge(sem_vs, 4)
    V.reciprocal(bn_sc[:, 6:8], bn_sc[:, 4:6]).then_inc(sem_bn, 1)
    S.wait_ge(sem_bn, 5)
    S.activation(bn_sc[:, 8:10], bn_sc[:, 6:8],
                 AF.Sqrt).then_inc(sem_ss, 1)
    V.wait_ge(sem_ss, 1)
    V.tensor_tensor(bn_sc[:, 6:8], bn_sc[:, 8:10], gam[:, :],
                    ALU.mult).then_inc(sem_bn, 1)
    V.wait_ge(sem_bn, 6)
    V.tensor_tensor(bn_sc[:, 10:12], bn_sc[:, 6:8], chw[:, :],
                    ALU.mult).then_inc(sem_bn, 1)
    V.tensor_tensor(bn_sc[:, 2:4], bn_sc[:, 0:2], bn_sc[:, 6:8],
                    ALU.mult).then_inc(sem_bn, 1)
    V.wait_ge(sem_bn, 8)
    V.tensor_tensor(bn_sc[:, 2:4], bet[:, :], bn_sc[:, 2:4],
                    ALU.subtract).then_inc(sem_bn, 1)
    V.wait_ge(sem_bn, 9)
    V.tensor_tensor(bn_sc[:, 12:14], bn_sc[:, 2:4], chw[:, :],
                    ALU.mult).then_inc(sem_vs, 1)
    V.wait_ge(sem_vs, 5)

    # affine + residual in place in ysb_all, then store: three slices as
    # single fused affine_then_add ops on DVE, the fourth via ACT affine +
    # gpsimd add so it runs concurrently with DVE's chain.
    half = nn // 2
    S.wait_ge(sem_vs, 5)
    V.wait_ge(sem_vs, 5)
    ys1 = ysb_all[:, 1, slice(half, nn)]
    S.activation(ys1, ys1, AF.Identity, bias=bn_sc[:, 13:14],
                 scale=bn_sc[:, 11:12]).then_inc(sem_ss, 1)
    G.wait_ge(sem_ss, 2)
    G.tensor_tensor(ys1, ys1, xres[:, 1, slice(half, nn)], ALU.add)
    G.dma_start(out=out_d[P:2 * P, slice(half, nn)],
                in_=ys1).then_inc(sem_ds, 16)
    dma_eng = (Y, Y, S)
    for idx, (cc, h) in enumerate(((0, 0), (0, 1), (1, 0))):
        sl = slice(h * half, (h + 1) * half)
        yv = ysb_all[:, cc, sl]
        V.affine_then_add(yv, yv, xres[:, cc, sl],
                          bn_sc[:, 10 + cc:11 + cc],
                          bn_sc[:, 12 + cc:13 + cc]).then_inc(sem_vd, 1)
        dma_eng[idx].wait_ge(sem_vd, idx + 1)
        dma_eng[idx].dma_start(out=out_d[cc * P:(cc + 1) * P, sl],
                               in_=yv).then_inc(sem_ds, 16)
    Y.wait_ge(sem_ds, 64)

    nc.compile()
    return nc


_NC_CACHE = {}


def _get_nc():
    if "nc" not in _NC_CACHE:
        _NC_CACHE["nc"] = _build()
    return _NC_CACHE["nc"]


def _prep_inputs(x_b, theta_w, phi_w, g_w, out_w, bn_gamma, bn_beta,
                 fc1_w, fc1_b, fc2_w, fc2_b):
    bf = ml_dtypes.bfloat16
    return {
        "x": np.ascontiguousarray(x_b, dtype=np.float32),
        "wq_t": np.ascontiguousarray(np.asarray(theta_w, np.float32).T).astype(bf),
        "wk_t": np.ascontiguousarray(np.asarray(phi_w, np.float32).T).astype(bf),
        "wg_t": np.ascontiguousarray(np.asarray(g_w, np.float32).T).astype(bf),
        "wo_t": np.ascontiguousarray(np.asarray(out_w, np.float32).T).astype(bf),
        "fc1_wt": np.ascontiguousarray(
            (np.asarray(fc1_w, np.float32) / N).T).astype(np.float32),
        "fc1_b": np.ascontiguousarray(fc1_b, dtype=np.float32),
        "fc2_wt": np.ascontiguousarray(
            np.asarray(fc2_w, np.float32).T).astype(np.float32),
        "fc2_bn": np.ascontiguousarray(-np.asarray(fc2_b, np.float32)),
        "bn_gamma": np.ascontiguousarray(bn_gamma, dtype=np.float32),
        "bn_beta": np.ascontiguousarray(bn_beta, dtype=np.float32),
    }


def _run(inputs, trace=False):
    nc = _get_nc()
    x = np.asarray(inputs["x"], dtype=np.float32)
    xs = x.reshape(B, C, N)
    in_maps = [
        _prep_inputs(xs[i], inputs["theta_w"], inputs["phi_w"], inputs["g_w"],
                     inputs["out_w"], inputs["bn_gamma"], inputs["bn_beta"],
                     inputs["fc1_w"], inputs["fc1_b"], inputs["fc2_w"],
                     inputs["fc2_b"])
        for i in range(B)
    ]
    res = run_bass_kernel_spmd(nc, in_maps, list(range(B)), trace=trace)
    out = np.stack([np.asarray(res.results[i]["out"], dtype=np.float32)
                    for i in range(B)])
    return out.reshape(B, C, H, W), res


def kernel(**inputs) -> np.ndarray:
    out, _ = _run(inputs, trace=False)
    return out

